# revision 94
# baseline (speedup 1.0000x reference)
"""Trainium2 Bass kernel for dual-input complement-softmax attention (fp8).

Same algebra as the f32r baseline, but with fp8e4m3 operands everywhere on
the PE so the big matmuls (projections, VW, PV) run in DoubleRow perf mode
(256-deep contraction at 0.5 cycles/row = 4x f32r/bf16 throughput).

Scaling scheme (host folds into weights; relu commutes with positive scale):
  q' = 32*q_true (s_attn folded too), k' = 8*k, v' = 8*v, Wp' = 4*Wp
  dots_psum = 256*dots_true  -> P = exp(dots - 1.5) in e4m3, via ACT
  activation(Exp) for most tiles; every 4th tile of the phase-1 slabs runs an
  int8 Schraudolph on the DVE instead (one tensor_scalar whose int8 output
  bits ARE the e4m3 encoding), keeping the ACT exp stream the only pacer.
  (real dots lie in [0.1, 6.9]; P in [0.25, 230])
  VW = v'@Wp'^T = 32*VW_true (e4m3, |VW|max ~ 440 < 448)
  out = relu((h - F/r)/32), descale folded into the final DVE ops.
h is computed from f32 sums of the quantized v and an f32 copy of the
*dequantized* fp8 Wp so the dominant h term carries no fp8 weight error.

Sharding: 8 cores = 4 batches x 2 query-row halves, no collectives.
Output is written n-major [NQ, 256] bf16; host transposes and upcasts.
"""

import numpy as np
import ml_dtypes

B, C, HH, WW = 4, 256, 64, 64
N = HH * WW        # 4096 keys per batch
NQ = N // 2        # 2048 query rows per core
INTER = 128
OUT = 256
NCORES = 8

AQ, AK, AV, AP_ = 32.0, 8.0, 8.0, 4.0
EXP_BIAS = -1.5
DOTS_DESCALE = 1.0 / (AQ * AK)
OUT_DESCALE = 1.0 / (AV * AP_)
# int8 Schraudolph for the offloaded tiles: int8(dots_psum*EXPA8 + EXPB8)
# bit-pattern, read as float8e4m3, approximates exp(dots_true - 1.5)
# (trunc semantics; B tuned numerically: max rel err ~8%, rms ~3.2% —
# the same class as e4m3(exp)).
EXPA8 = 8.0 / float(np.log(2.0)) / (AQ * AK)
EXPB8 = 38.73

_NC_CACHE = {}


def _build_nc():
    import concourse.bacc as bacc
    import concourse.mybir as mybir
    import concourse.tile as tile

    f32 = mybir.dt.float32
    f32r = mybir.dt.float32r
    fp8 = mybir.dt.float8e4
    i8 = mybir.dt.int8
    bf16 = mybir.dt.bfloat16
    A = mybir.AluOpType
    AF = mybir.ActivationFunctionType
    X = mybir.AxisListType.X
    XY = mybir.AxisListType.XY
    DR = mybir.MatmulPerfMode.DoubleRow

    nc = bacc.Bacc(None, target_bir_lowering=False)

    x12 = nc.dram_tensor("x12", [2 * C, N], fp8, kind="ExternalInput")
    # DoubleRow weight pairs: [cin128, proj(q,k,v), pair, cout128]
    wqkv = nc.dram_tensor("wqkv", [128, 3, 2, 128], fp8, kind="ExternalInput")
    wp8 = nc.dram_tensor("wp8", [128, 2, OUT], fp8, kind="ExternalInput")
    wph = nc.dram_tensor("wph", [128, 2, OUT], f32, kind="ExternalInput")
    bqkv = nc.dram_tensor("bqkv", [3 * INTER, 1], f32, kind="ExternalInput")
    bp_row = nc.dram_tensor("bp_row", [1, OUT], f32, kind="ExternalInput")

    out = nc.dram_tensor("out", [NQ, OUT], bf16, kind="ExternalOutput")

    MCHUNKS = N // 128           # 32 key chunks
    SB = 512                     # query superblock
    NSB = NQ // SB               # 4

    with tile.TileContext(nc) as tc:
        with (
            tc.tile_pool(name="persist", bufs=1) as persist,
            tc.tile_pool(name="ep", bufs=60) as ep,
            tc.tile_pool(name="zp", bufs=8) as zp,
            tc.tile_pool(name="ps", bufs=3, space="PSUM") as ps,
            tc.tile_pool(name="sm", bufs=2, space="PSUM") as sm,
        ):
            # ---- persistent tiles ----
            x_sb = persist.tile([128, 4, N], fp8)   # [c, (x1|x2 pairs), m]
            wqkv_sb = persist.tile([128, 3, 2, 128], fp8)
            wp_sb = persist.tile([128, 2, OUT], fp8)
            wph_sb = persist.tile([128, 2, OUT], f32r)
            bqkv_sb = persist.tile([128, 3], f32)
            bp_sb = persist.tile([1, OUT], f32)
            k2_sb = persist.tile([128, N], fp8)              # [c, m]
            q_sb = persist.tile([128, NQ], fp8)              # [c, n]
            v_sb = persist.tile([128, MCHUNKS, 2, 128], fp8)  # [c, j, v2|v1, m]
            vw_sb = persist.tile([128, MCHUNKS, 258], fp8)   # [m%128, j, o|1,1]
            h_row = persist.tile([1, OUT], f32)
            h_bc = persist.tile([128, OUT], f32)
            ebias = persist.tile([128, 1], f32)
            scr = persist.tile([128, 1], f32)

            def xload(s2):
                msl = slice(s2 * 1024, (s2 + 1) * 1024)
                nc.sync.dma_start(
                    x_sb[:, :, msl],
                    x12[:, msl].rearrange("(a p) n -> p a n", p=128))

            # ---- DMA front first: x slabs lead the SP HWDGE queue; the
            # small weight tensors ride the idle Pool SWDGE queue ----
            nc.gpsimd.dma_start(bqkv_sb[:], bqkv[:].rearrange("(a p) o -> p (a o)", p=128))
            nc.sync.dma_start(wqkv_sb[:], wqkv[:])
            for hq in (0, 1):
                qsl = slice(hq * 512, (hq + 1) * 512)
                nc.sync.dma_start(
                    x_sb[:, :, qsl],
                    x12[:, qsl].rearrange("(a p) n -> p a n", p=128))
            xload(1)
            xload(2)
            xload(3)
            nc.sync.dma_start(wp_sb[:], wp8[:])
            nc.sync.dma_start(wph_sb[:], wph[:].bitcast(f32r))
            nc.sync.dma_start(bp_sb[:], bp_row[:])

            nc.vector.memset(ebias[:], EXP_BIAS)
            nc.vector.memset(scr[:], 0.0)
            nc.vector.memset(vw_sb[:, :, 256:258], 1.0)
            # dummy activation: absorbs the ACT table load at t~0
            nc.scalar.activation(scr[:], scr[:], AF.Relu, bias=ebias[:])

            def proj_half(a, xoff, s2, hf, dst, bias, act=False, pool=None):
                """Project half hf of slab s2 of x (xoff 0 = x1, 2 = x2)
                with weight set a; relu+bias into dst.  act=True runs the
                relu on the ACT engine (ramp window).  pool overrides the
                psum pool (slab 0 borrows the still-empty dps slots so the
                k/q projections aren't serialized by the 2-buf rotation)."""
                o = s2 * 1024 + hf * 512
                if pool == "ps":
                    psum = ps.tile([128, 512], f32, tag="ps", name="pp0")
                else:
                    psum = sm.tile([128, 512], f32, tag="small", name="pp")
                nc.tensor.matmul(psum[:], wqkv_sb[:, a],
                                 x_sb[:, xoff:xoff + 2, o:o + 512],
                                 start=True, stop=True, perf_mode=DR)
                if act:
                    nc.scalar.activation(dst, psum[:], AF.Relu, bias=bias)
                else:
                    nc.vector.tensor_scalar(dst, psum[:], bias,
                                            0.0, A.add, A.max)

            def proj_relu(a, xoff, s2, dst_half, bias, act=False):
                for hf in (0, 1):
                    proj_half(a, xoff, s2, hf, dst_half(hf), bias, act=act)

            def vw_mms(s2):
                """8 VW chunks for the 1024-wide m-slab s2 (DoubleRow pairs),
                two chunks per psum tile to batch the DVE copies."""
                for mc in range(0, 8, 2):
                    j = s2 * 8 + mc
                    vwps = sm.tile([128, 512], f32, tag="small", name="vwps")
                    for d in (0, 1):
                        nc.tensor.matmul(vwps[:, d * 256:(d + 1) * 256],
                                         v_sb[:, j + d], wp_sb[:],
                                         start=True, stop=True, perf_mode=DR)
                    nc.vector.tensor_copy(
                        vw_sb[:, j:j + 2, 0:256],
                        vwps[:].rearrange("p (d o) -> p d o", d=2))

            def v1_block(s2, with_q):
                if with_q:
                    proj_relu(0, 0, s2,
                              lambda hf: q_sb[:, s2 * 1024 + hf * 512:
                                              s2 * 1024 + (hf + 1) * 512],
                              bqkv_sb[:, 0:1])
                proj_relu(2, 0, s2,
                          lambda hf: v_sb[:, 8 * s2 + 4 * hf:
                                          8 * s2 + 4 * hf + 4, 1, :],
                          bqkv_sb[:, 2:3])
                vw_mms(s2)

            exp_map = {sb: [None] * (MCHUNKS // 2) for sb in range(NSB)}

            def dots_slab(sbs, s2, jls=(0, 1, 2, 3)):
                """dps+exp for m-chunks of slab s2, for each query superblock.
                Every 4th tile (sb>=2) bypasses the ACT engine via the int8
                Schraudolph on the DVE."""
                for sb in sbs:
                    nsl = slice(sb * SB, (sb + 1) * SB)
                    for jl in jls:
                        jj = s2 * 4 + jl
                        dps = ps.tile([128, 1024], f32, tag="ps", name="dps")
                        for u in (0, 1):
                            j = jj * 2 + u
                            nc.tensor.matmul(dps[:, u * 512:(u + 1) * 512],
                                             k2_sb[:, j * 128:(j + 1) * 128],
                                             q_sb[:, nsl], start=True, stop=True)
                        et = ep.tile([128, 1024], fp8, tag="exp", name="et")
                        if jl == 3 and sb >= 2:
                            # int8 Schraudolph: the int8 bits ARE the e4m3
                            # encoding of ~exp(dots-1.5); one DVE op, no
                            # Pool convert pass.
                            nc.vector.tensor_scalar(et[:].bitcast(i8), dps[:],
                                                    EXPA8, EXPB8,
                                                    A.mult, A.add)
                        else:
                            nc.scalar.activation(et[:], dps[:], AF.Exp,
                                                 bias=ebias[:],
                                                 scale=DOTS_DESCALE)
                        exp_map[sb][jj] = et

            fps_map = {}    # (sb, t) -> (fps tile, n pairs accumulated)

            def pv_mms(sb, trange, jjs, fin):
                tiles = exp_map[sb]
                for t in trange:
                    fps, done = fps_map.get((sb, t), (None, 0))
                    if fps is None:
                        fps = sm.tile([128, 258], f32, tag="small", name="fps")
                    for jj in jjs:
                        etp = tiles[jj][:].rearrange("p (u n) -> p u n", u=2)
                        nc.tensor.matmul(fps[:], etp[:, :, t * 128:(t + 1) * 128],
                                         vw_sb[:, 2 * jj:2 * jj + 2, 0:258],
                                         start=(jj == 0),
                                         stop=(fin and jj == jjs[-1]),
                                         perf_mode=DR, skip_group_check=True)
                    fps_map[(sb, t)] = (fps, done + len(jjs))

            def pv_fin(sb, trange):
                for t in trange:
                    _, done = fps_map.get((sb, t), (None, 0))
                    pv_mms(sb, (t,), range(done, 16), True)

            def pv_ladder(sb, trange):
                for t in trange:
                    nt = sb * 4 + t
                    fps, _ = fps_map.pop((sb, t))
                    rn = zp.tile([128, 1], f32, tag="rn")
                    nc.vector.reciprocal(rn[:], fps[:, 256:257])
                    t2 = zp.tile([128, OUT], bf16, tag="t2")
                    nc.vector.scalar_tensor_tensor(t2[:], fps[:, 0:256], rn[:],
                                                   h_bc[:], A.mult, A.subtract)
                    z = zp.tile([128, OUT], bf16, tag="z")
                    zeng = nc.vector if sb == 3 else nc.gpsimd
                    zeng.tensor_scalar(z[:], t2[:], -OUT_DESCALE, 0.0,
                                       A.mult, A.max)
                    nc.sync.dma_start(out[nt * 128:(nt + 1) * 128, :], z[:])

            def pv_out(sb, trange=None):
                if trange is None:
                    trange = range(SB // 128)
                for t in trange:
                    pv_fin(sb, (t,))
                    pv_ladder(sb, (t,))

            # ---- phase 0: projections + VW, slab-local dots ----
            # Half-granular start: the first dots tiles fire right after the
            # first k/q half-relus (k on ACT in its pre-exp window, q on the
            # idle DVE), so the exp stream starts as early as possible.
            def kslc(s2, hf):
                return k2_sb[:, s2 * 1024 + hf * 512:s2 * 1024 + (hf + 1) * 512]

            for s2 in range(4):
                proj_relu(1, 2, s2,
                          lambda hf: kslc(s2, hf),
                          bqkv_sb[:, 1:2], act=(s2 == 0))
                if s2 == 0:
                    proj_relu(0, 0, 0,
                              lambda hf: q_sb[:, hf * 512:(hf + 1) * 512],
                              bqkv_sb[:, 0:1], act=True)
                else:
                    dots_slab((0,), s2 - 1)
                proj_relu(2, 2, s2,
                          lambda hf: v_sb[:, 8 * s2 + 4 * hf:
                                          8 * s2 + 4 * hf + 4, 0, :],
                          bqkv_sb[:, 2:3])
                if s2 > 0:
                    dots_slab((1,), s2 - 1)
                v1_block(s2, with_q=(s2 == 1))
            dots_slab((0, 1), 3)

            # ---- sumv totals, h ----
            sv2f = zp.tile([128, 1], f32, tag="svf")
            sv1f = zp.tile([128, 1], f32, tag="svf")
            nc.vector.tensor_reduce(sv2f[:], v_sb[:, :, 0, :], XY, A.add)
            nc.vector.tensor_reduce(sv1f[:], v_sb[:, :, 1, :], XY, A.add)
            hps = sm.tile([128, 258], f32, tag="small", name="hps")
            nc.tensor.matmul(hps[0:1, 0:256], sv2f[:], wph_sb[:, 0].bitcast(f32),
                             start=True, stop=False)
            nc.tensor.matmul(hps[0:1, 0:256], sv1f[:], wph_sb[:, 1].bitcast(f32),
                             start=False, stop=True)
            nc.vector.tensor_tensor(h_row[:], hps[0:1, 0:256], bp_sb[:], A.add)
            nc.gpsimd.partition_broadcast(h_bc[:], h_row[:])

            # ---- phase 1 steady state: PV tiles interleave with dots.  All
            # sb1 out-tiles drain early in loop 2 (their exps finished in
            # phase 0); sb2's drain as soon as loop 1 completes, so after the
            # final dots call only sb3 remains, pre-accumulated to pairs
            # 0..11 with a rolling fps handoff (t0->t2, t1->t3) ----
            for s2 in range(4):
                dots_slab((2,), s2)
                pv_out(0, (s2,))
            for s2 in range(3):
                dots_slab((3,), s2)
                pv_out(1, (s2,))
                pv_out(2, (s2,))
            # last slab: close the sb1/sb2 tiles before the final dots call
            # (their PV matmuls don't depend on it); their out-ladders then
            # overlap the final exp drain, and the sb3 pre-accumulation rolls
            # through the freed fps buffers
            dots_slab((3,), 3)
            pv_out(1, (3,))
            pv_out(2, (3,))
            pv_mms(3, (0, 1), range(0, 12), False)
            pv_out(3)

    nc.compile()
    return nc


def _host_prep(inputs):
    E4 = ml_dtypes.float8_e4m3fn
    s_attn = np.float32(INTER ** -0.5)
    x1 = np.asarray(inputs["x1"], np.float32).reshape(B, C, N)
    x2 = np.asarray(inputs["x2"], np.float32).reshape(B, C, N)
    x1_8 = x1.astype(E4)
    x2_8 = x2.astype(E4)

    def eff(Wn, bn, sn, tn, extra=np.float32(1.0)):
        Wm = np.asarray(inputs[Wn], np.float32)
        bb = np.asarray(inputs[bn], np.float32)
        ss = np.asarray(inputs[sn], np.float32)
        tt = np.asarray(inputs[tn], np.float32)
        W_eff = (ss[:, None] * Wm) * extra
        b_eff = (ss * bb + tt) * extra
        return np.ascontiguousarray(W_eff.T), b_eff   # W_eff.T: [cin, cout]

    wqT, bqe = eff("Wq", "bq", "sq", "tq", s_attn * np.float32(AQ))
    wkT, bke = eff("Wk", "bk", "sk", "tk", np.float32(AK))
    wvT, bve = eff("Wv", "bv", "sv", "tv", np.float32(AV))
    wpT, bpe = eff("Wp", "bp", "sp", "tp", np.float32(AP_))

    # DoubleRow pair layout [cin128, proj, pair, cout]
    wqkv8 = np.stack([w.reshape(2, 128, 128).transpose(1, 0, 2)
                      for w in (wqT, wkT, wvT)], axis=1).astype(E4)
    wpT_pair = wpT.reshape(2, 128, OUT).transpose(1, 0, 2)
    wp8 = wpT_pair.astype(E4)
    wph = np.ascontiguousarray(wpT_pair, np.float32)   # true f32 Wp for h

    common = dict(
        wqkv=np.ascontiguousarray(wqkv8),
        wp8=np.ascontiguousarray(wp8),
        wph=np.ascontiguousarray(wph),
        bqkv=np.concatenate([bqe, bke, bve]).reshape(3 * INTER, 1),
        bp_row=(bpe * np.float32(AV)).reshape(1, OUT),
    )
    in_maps = []
    for c in range(NCORES):
        b, half = c // 2, c % 2
        # m-axis permutation: own query half first (identical for x1 and x2,
        # so all sum-over-m quantities are unchanged)
        perm = (np.r_[NQ:N, 0:NQ] if half else np.r_[0:N]).astype(np.intp)
        in_maps.append(dict(
            x12=np.ascontiguousarray(
                np.concatenate([x1_8[b][:, perm], x2_8[b][:, perm]], axis=0)),
            **common,
        ))
    return in_maps


def kernel(**inputs):
    from concourse.bass_utils import run_bass_kernel_spmd

    if "nc" not in _NC_CACHE:
        _NC_CACHE["nc"] = _build_nc()
    nc = _NC_CACHE["nc"]

    in_maps = _host_prep(inputs)
    res = run_bass_kernel_spmd(nc, in_maps, core_ids=list(range(NCORES)))

    full = np.empty((B, OUT, N), dtype=np.float32)
    for c in range(NCORES):
        b, half = c // 2, c % 2
        full[b][:, half * NQ:(half + 1) * NQ] = \
            res.results[c]["out"].astype(np.float32).T
    return full.reshape(B, OUT, HH, WW)


if __name__ == "__main__":
    rng = np.random.default_rng(0)
    fake = {}
    fake["x1"] = rng.standard_normal((B, C, HH, WW), dtype=np.float32)
    fake["x2"] = rng.standard_normal((B, C, HH, WW), dtype=np.float32)
    for k, oc in (("q", INTER), ("k", INTER), ("v", INTER), ("p", OUT)):
        ic = C if k != "p" else 2 * INTER
        fake["W" + k] = rng.standard_normal((oc, ic), dtype=np.float32) * ic ** -0.5
        fake["b" + k] = np.zeros(oc, np.float32)
        fake["s" + k] = rng.uniform(0.5, 1.5, oc).astype(np.float32)
        fake["t" + k] = rng.standard_normal(oc, dtype=np.float32) * 0.1
    o = kernel(**fake)
    print("kernel ran, out shape", o.shape)
